# revision 17
# baseline (speedup 1.0000x reference)
"""MoE layer (router + top-2 experts + shared expert) on 8 TRN2 NeuronCores.

Strategy (expert-parallel, sparse):
  - Each core owns one expert e: receives gate_w[e]/up_w[e]/down_w[e].
  - Router is replicated: logits computed in compensated bf16 (x and
    router_w each split into hi+lo bf16 parts) so the top-2 selection
    matches the fp32 reference.
  - Each core compacts the tokens routed to its expert (capacity C),
    runs the expert MLP on the compact set in bf16, scales rows by the
    renormalized top-2 weight, and scatter-adds into a [T, H] partial.
  - Shared expert is sharded along the intermediate dim (I/8 per core)
    and computed for all tokens into the same partial.
  - ReduceScatter(add) over the 8 cores; core i keeps token rows
    [i*T/8, (i+1)*T/8); host concatenates the slices.
"""

import sys

sys.path.insert(0, "/opt/trn_rl_repo")

from contextlib import ExitStack
from dataclasses import dataclass

import numpy as np

import concourse.bass as bass
import concourse.mybir as mybir
import concourse.tile as tile
from concourse import bacc, bass_utils

F32 = mybir.dt.float32
BF16 = mybir.dt.bfloat16
I32 = mybir.dt.int32
P = 128
BIG = 1.0e5  # OOB slot marker (skipped via bounds_check; BIG*H must fit int32)


@dataclass(frozen=True)
class Cfg:
    T: int = 2048  # tokens (B*S)
    H: int = 2048  # hidden
    I: int = 1408  # expert intermediate
    E: int = 8  # experts
    C: int = 768  # per-expert token capacity (>= max expert load)
    NC: int = 8  # cores

    @property
    def TC(self):
        return self.T // P

    @property
    def HC(self):
        return self.H // P

    @property
    def IC(self):
        return self.I // P

    @property
    def CC(self):
        return self.C // P

    @property
    def ISL(self):
        return self.I // self.NC  # shared-expert I slice per core

    @property
    def TO(self):
        return self.T // self.NC  # output token rows per core


def _blocks(total, size):
    return [(s, min(size, total - s)) for s in range(0, total, size)]


def build_moe(nc, cfg: Cfg):
    """Emit the full per-core program (SPMD: identical on all cores)."""
    T, H, II, E = cfg.T, cfg.H, cfg.I, cfg.E
    ISL, TO, TC = cfg.ISL, cfg.TO, cfg.TC

    # ---- kernel I/O ----
    x_in = nc.dram_tensor("x", [T, H], F32, kind="ExternalInput")
    rw_in = nc.dram_tensor("rw", [E, H], F32, kind="ExternalInput")
    wg_in = nc.dram_tensor("wg", [II, H], F32, kind="ExternalInput")
    wu_in = nc.dram_tensor("wu", [II, H], F32, kind="ExternalInput")
    wd_in = nc.dram_tensor("wd", [H, II], F32, kind="ExternalInput")
    wsg_in = nc.dram_tensor("wsg", [ISL, H], F32, kind="ExternalInput")
    wsu_in = nc.dram_tensor("wsu", [ISL, H], F32, kind="ExternalInput")
    wsd_in = nc.dram_tensor("wsd", [H, ISL], F32, kind="ExternalInput")
    sel_in = nc.dram_tensor("sel", [P, E], F32, kind="ExternalInput")
    out_ext = nc.dram_tensor("out", [TO, H], F32, kind="ExternalOutput")

    # ---- compile-time constants (packed into the NEFF) ----
    ut_np = np.triu(np.ones((P, P), dtype=np.float32))  # ut[k, m] = k <= m
    ids_np = (np.arange(TC)[None, :] * P + np.arange(P)[:, None]).astype(np.int32)
    ut_dram = nc.inline_tensor(ut_np, name="ut_const")
    ids_dram = nc.inline_tensor(ids_np, name="ids_const")
    eye_dram = nc.inline_tensor(np.eye(P, dtype=np.float32), name="eye_const")
    ones_dram = nc.inline_tensor(np.ones((1, P), dtype=np.float32), name="ones_const")

    with tile.TileContext(nc) as tc:
        _emit(tc, cfg, x_in, rw_in, wg_in, wu_in, wd_in, wsg_in, wsu_in,
              wsd_in, sel_in, out_ext, ut_dram, ids_dram, eye_dram, ones_dram)
    return nc


def _emit(tc, cfg: Cfg, x_in, rw_in, wg_in, wu_in, wd_in, wsg_in, wsu_in,
          wsd_in, sel_in, out_ext, ut_dram, ids_dram, eye_dram, ones_dram):
    nc = tc.nc
    T, H, II, E, C = cfg.T, cfg.H, cfg.I, cfg.E, cfg.C
    TC, HC, IC, CC, ISL, TO = cfg.TC, cfg.HC, cfg.IC, cfg.CC, cfg.ISL, cfg.TO
    NB_T = _blocks(T, 512)
    NB_H = _blocks(H, 512)
    NB_C = _blocks(C, 512)
    add = mybir.AluOpType.add
    sub = mybir.AluOpType.subtract
    mult = mybir.AluOpType.mult
    is_eq = mybir.AluOpType.is_equal
    is_gt = mybir.AluOpType.is_gt
    AF = mybir.ActivationFunctionType

    ctx = ExitStack()  # whole-kernel pools
    ctx1 = ExitStack()  # router/compaction-phase pools (released first)
    ctx3 = ExitStack()  # expert-phase pools (created after ctx1 closes)

    # Pools reserve space statically from creation to release, LIFO order.
    # PSUM budget: phase1 ps_sh(4) + ps_r(2) = 6; phase3 ps_sh(4) + ps_g(2)
    # + ps_eo(2) = 8 banks.
    consts = ctx.enter_context(tc.tile_pool(name="consts", bufs=1))
    dram = ctx.enter_context(tc.tile_pool(name="dram", bufs=1, space="DRAM"))
    ps_sh = ctx.enter_context(tc.tile_pool(name="ps_sh", bufs=4, space="PSUM"))
    pw = ctx.enter_context(tc.tile_pool(name="pw", bufs=3))
    psg = ctx.enter_context(tc.tile_pool(name="psg", bufs=IC))
    psmid = ctx.enter_context(tc.tile_pool(name="psmid", bufs=1))
    pev = ctx.enter_context(tc.tile_pool(name="pev", bufs=2))
    pshw = ctx.enter_context(tc.tile_pool(name="pshw", bufs=2 * HC))
    pxs = ctx.enter_context(tc.tile_pool(name="pxs", bufs=4))

    ps_r = ctx1.enter_context(tc.tile_pool(name="ps_r", bufs=2, space="PSUM"))
    px = ctx1.enter_context(tc.tile_pool(name="px", bufs=2))
    pxc = ctx1.enter_context(tc.tile_pool(name="pxc", bufs=2))
    pxtl = ctx1.enter_context(tc.tile_pool(name="pxtl", bufs=HC + 2))
    prt = ctx1.enter_context(tc.tile_pool(name="prt", bufs=2))
    prw = ctx1.enter_context(tc.tile_pool(name="prw", bufs=HC))
    pmeta = ctx1.enter_context(tc.tile_pool(name="pmeta", bufs=2))

    # ---------------- DRAM scratch ----------------
    xbf_d = dram.tile([T, H], BF16)  # bf16 copy of x
    compact_d = dram.tile([C, H], BF16)  # gathered tokens for this expert
    combc_d = dram.tile([CC, P], F32)  # per-slot combine weight
    tokpos_d = dram.tile([CC, P], I32)  # per-slot source token id
    outp_d = dram.tile([T, H], BF16)  # this core's partial output
    rs_d = dram.tile([TO, H], BF16)  # reduce-scatter result
    RW = 32 + E  # packed router-weight rows: hi at 0, lo at 32 (16-aligned)
    rwcat_d = dram.tile([48, H], BF16)

    # ---------------- long-lived consts ----------------
    combc_sb = consts.tile([P, CC], F32)
    tokpos_sb = consts.tile([P, CC], I32)

    # ---------------- phase-1 consts ----------------
    ut_sb = prt.tile([P, P], F32, bufs=1)
    nc.sync.dma_start(ut_sb[:], ut_dram[:])
    eye_sb = prt.tile([P, P], F32, bufs=1)
    nc.sync.dma_start(eye_sb[:], eye_dram[:])
    ones_sb = prt.tile([1, P], F32, bufs=1)
    nc.sync.dma_start(ones_sb[:], ones_dram[:])
    onescol_sb = prt.tile([P, 1], F32, bufs=1)
    nc.vector.memset(onescol_sb[:], 1.0)
    ids_sb = prt.tile([P, TC], I32, bufs=1)
    nc.sync.dma_start(ids_sb[:], ids_dram[:])
    sel_sb = prt.tile([P, E], F32, bufs=1)
    nc.sync.dma_start(sel_sb[:], sel_in[:])

    # ---------------- router weights: hi/lo split, transposed ----------------
    # rw_cat rows [0:E] = bf16(rw), rows [E:2E] = bf16(rw - bf16(rw))
    rw_f = prt.tile([E, H], F32, bufs=1)
    nc.sync.dma_start(rw_f[:], rw_in[:])
    rw_hi = prt.tile([E, H], BF16, bufs=1)
    nc.vector.tensor_copy(rw_hi[:], rw_f[:])
    rw_lo = prt.tile([E, H], BF16, bufs=1)
    nc.vector.tensor_tensor(rw_lo[:], rw_f[:], rw_hi[:], op=sub)
    zrw = prt.tile([48, H], BF16, bufs=1)
    nc.vector.memset(zrw[:], 0.0)
    nc.sync.dma_start(rwcat_d[:], zrw[:])
    nc.sync.dma_start(rwcat_d[0:E, :], rw_hi[:])
    nc.sync.dma_start(rwcat_d[32:32 + E, :], rw_lo[:])
    # transpose -> per-h-tile [128, 48] (cols 0:E = hi, 32:32+E = lo, rest 0)
    rwt = []
    for h in range(HC):
        t = prw.tile([P, 48], BF16, tag="rwt")
        nc.sync.dma_start(t[:], rwcat_d[:, h * P:(h + 1) * P], transpose=True)
        rwt.append(t)

    # ------- stage A: x chunks -> bf16 DRAM copy + router correction -------
    logits_sb = prt.tile([E, T], F32, bufs=1)  # accumulated logits^T

    for t in range(TC):
        xf = px.tile([P, H], F32, tag="xf")
        nc.sync.dma_start(xf[:], x_in[t * P:(t + 1) * P, :])
        xh = pxc.tile([P, H], BF16, tag="xh")
        nc.vector.tensor_copy(xh[:], xf[:])
        nc.sync.dma_start(xbf_d[t * P:(t + 1) * P, :], xh[:])
        xl = pxc.tile([P, H], BF16, tag="xl")
        nc.vector.tensor_tensor(xl[:], xf[:], xh[:], op=sub)
        # router correction for this token chunk: x_lo @ rw_hi^T -> [E, 128]
        ps_b = ps_r.tile([E, P], F32, space="PSUM", tag="r")
        for h in range(HC):
            xtl = pxtl.tile([P, P], BF16, tag="xtl")
            nc.sync.dma_start(xtl[:], xl[:, h * P:(h + 1) * P], transpose=True)
            nc.tensor.matmul(ps_b[:], rwt[h][:, 0:E], xtl[:],
                             start=(h == 0), stop=(h == HC - 1))
        nc.vector.tensor_copy(logits_sb[:, t * P:(t + 1) * P], ps_b[:])

    # --- combined xT stream: router pass A + shared-expert GEMM1/2 ---
    # One transposed read of x_bf feeds the router logits matmul AND the
    # shared expert gate/up matmuls (sharded along I: this core's slice).
    msl = _blocks(ISL, P)  # m-chunks of the shared I slice

    def shared_wt(w_in, label):
        wt = [pshw.tile([P, ISL], BF16, tag="shwt", name=f"{label}{h}")
              for h in range(HC)]
        for (m0, mm) in msl:
            wn = pw.tile([P, H], BF16, tag="w")
            nc.gpsimd.dma_start(wn[0:mm, :], w_in[m0:m0 + mm, :])
            for h in range(HC):
                nc.sync.dma_start(wt[h][:, m0:m0 + mm],
                                  wn[0:mm, h * P:(h + 1) * P], transpose=True)
        return wt

    wsgt = shared_wt(wsg_in, "wsgt")
    wsut = shared_wt(wsu_in, "wsut")
    smid = [psmid.tile([min(P, ISL - m0), T], BF16, tag=f"smid{mi}",
                       name=f"smid{mi}")
            for mi, (m0, mm) in enumerate(msl)]

    for (n0, nn) in NB_T:
        ps_a = ps_r.tile([48, 512], F32, space="PSUM", tag="r")
        pgs = [ps_sh.tile([P, 512], F32, space="PSUM", tag="sh",
                          name=f"pgs{n0}_{mi}") for mi in range(len(msl))]
        pus = [ps_sh.tile([P, 512], F32, space="PSUM", tag="sh",
                          name=f"pus{n0}_{mi}") for mi in range(len(msl))]
        for h in range(HC):
            xt = pxs.tile([P, 512], BF16, tag="xt")
            nc.sync.dma_start(xt[:, 0:nn], xbf_d[n0:n0 + nn, h * P:(h + 1) * P],
                              transpose=True)
            nc.tensor.matmul(ps_a[:, 0:nn], rwt[h][:], xt[:, 0:nn],
                             start=(h == 0), stop=(h == HC - 1))
            for mi, (m0, mm) in enumerate(msl):
                nc.tensor.matmul(pgs[mi][0:mm, 0:nn], wsgt[h][:, m0:m0 + mm],
                                 xt[:, 0:nn], start=(h == 0), stop=(h == HC - 1))
                nc.tensor.matmul(pus[mi][0:mm, 0:nn], wsut[h][:, m0:m0 + mm],
                                 xt[:, 0:nn], start=(h == 0), stop=(h == HC - 1))
        tmp = prt.tile([E, 512], F32, tag="la")
        nc.vector.tensor_copy(tmp[:, 0:nn], ps_a[0:E, 0:nn])
        nc.vector.tensor_tensor(tmp[:, 0:nn], tmp[:, 0:nn],
                                ps_a[32:32 + E, 0:nn], op=add)
        nc.vector.tensor_tensor(logits_sb[:, n0:n0 + nn], logits_sb[:, n0:n0 + nn],
                                tmp[:, 0:nn], op=add)
        for mi, (m0, mm) in enumerate(msl):
            sig = pev.tile([P, 512], BF16, tag="sig")
            nc.scalar.activation(sig[0:mm, 0:nn], pgs[mi][0:mm, 0:nn], AF.Sigmoid)
            nc.vector.tensor_tensor(smid[mi][:, n0:n0 + nn], pgs[mi][0:mm, 0:nn],
                                    sig[0:mm, 0:nn], op=mult)
            nc.vector.tensor_tensor(smid[mi][:, n0:n0 + nn], pus[mi][0:mm, 0:nn],
                                    smid[mi][:, n0:n0 + nn], op=mult)

    # ---------------- router epilogue (token-major) ----------------
    # PE-transpose logits^T [E, T] into [128, TC*E]
    ps_lt = ps_r.tile([P, TC * E], F32, space="PSUM", tag="r")
    for t in range(TC):
        nc.tensor.transpose(ps_lt[:, t * E:(t + 1) * E],
                            logits_sb[:, t * P:(t + 1) * P], eye_sb[0:E, 0:E])
    ltok = prt.tile([P, TC * E], F32, tag="ltok")
    nc.vector.tensor_copy(ltok[:], ps_lt[:])

    def v3(ap_tile):  # [P, TC*E] -> [P, TC, E]
        return ap_tile[:].rearrange("p (c e) -> p c e", e=E)

    l3 = v3(ltok)
    m1 = prt.tile([P, TC], F32, tag="m1")
    nc.vector.tensor_reduce(m1[:], l3, mybir.AxisListType.X, mybir.AluOpType.max)
    m1b = m1[:].unsqueeze(2).to_broadcast([P, TC, E])
    eq1_t = prt.tile([P, TC * E], F32, tag="eq1")
    nc.vector.tensor_tensor(v3(eq1_t), l3, m1b, op=is_eq)
    lm_t = prt.tile([P, TC * E], F32, tag="lm")
    nc.vector.tensor_scalar(lm_t[:], eq1_t[:], -1.0e30, None, op0=mult)
    nc.vector.tensor_tensor(lm_t[:], lm_t[:], ltok[:], op=add)
    m2 = prt.tile([P, TC], F32, tag="m2")
    nc.vector.tensor_reduce(m2[:], v3(lm_t), mybir.AxisListType.X,
                            mybir.AluOpType.max)
    # top-2 mask = (l == m1) | (masked == m2)
    mask2_t = prt.tile([P, TC * E], F32, tag="mask2")
    nc.vector.tensor_tensor(v3(mask2_t), v3(lm_t),
                            m2[:].unsqueeze(2).to_broadcast([P, TC, E]), op=is_eq)
    nc.vector.tensor_tensor(mask2_t[:], mask2_t[:], eq1_t[:], op=add)
    # w = exp(l - m1) * mask / (1 + exp(m2 - m1))
    es_t = prt.tile([P, TC * E], F32, tag="es")
    nc.vector.tensor_tensor(v3(es_t), l3, m1b, op=sub)
    nc.scalar.activation(es_t[:], es_t[:], AF.Exp)
    dd = prt.tile([P, TC], F32, tag="dd")
    nc.vector.tensor_tensor(dd[:], m2[:], m1[:], op=sub)
    nc.scalar.activation(dd[:], dd[:], AF.Exp)
    nc.vector.tensor_scalar_add(dd[:], dd[:], 1.0)
    rcp = prt.tile([P, TC], F32, tag="rcp")
    nc.vector.reciprocal(rcp[:], dd[:])
    nc.vector.tensor_tensor(es_t[:], es_t[:], mask2_t[:], op=mult)
    nc.vector.tensor_tensor(v3(es_t), v3(es_t),
                            rcp[:].unsqueeze(2).to_broadcast([P, TC, E]), op=mult)
    # select this core's expert column; comb_all[p, c]
    selb = sel_sb[:].unsqueeze(1).to_broadcast([P, TC, E])
    wsel = prt.tile([P, TC * E], F32, tag="wsel")
    nc.vector.tensor_tensor(v3(wsel), v3(es_t), selb, op=mult)
    comb_all = prt.tile([P, TC], F32, bufs=1)
    nc.vector.tensor_reduce(comb_all[:], v3(wsel), mybir.AxisListType.X, add)
    mask_all = prt.tile([P, TC], F32, bufs=1)
    nc.vector.tensor_scalar(mask_all[:], comb_all[:], 0.0, None, op0=is_gt)

    # ---------------- compaction ----------------
    ps_cs = ps_r.tile([P, TC], F32, space="PSUM", tag="r")
    nc.tensor.matmul(ps_cs[:], ut_sb[:], mask_all[:], start=True, stop=True)
    cs_sb = pmeta.tile([P, TC], F32, tag="cs_sb")
    nc.vector.tensor_copy(cs_sb[:], ps_cs[:])
    # column totals via ones-vector matmul (no partition-127 access allowed)
    ps_ct = ps_r.tile([1, TC], F32, space="PSUM", tag="r")
    nc.tensor.matmul(ps_ct[:], onescol_sb[:], mask_all[:], start=True, stop=True)
    colsum = pmeta.tile([1, TC], F32, tag="colsum")
    nc.vector.tensor_copy(colsum[:], ps_ct[:])
    offs = pmeta.tile([1, TC], F32, tag="offs")
    nc.vector.memset(offs[:, 0:1], 0.0)
    for c in range(1, TC):
        nc.vector.tensor_tensor(offs[:, c:c + 1], offs[:, c - 1:c],
                                colsum[:, c - 1:c], op=add)
    # broadcast offs over partitions via a K=1 outer-product matmul
    ps_of = ps_r.tile([P, TC], F32, space="PSUM", tag="r")
    nc.tensor.matmul(ps_of[:], ones_sb[:], offs[:], start=True, stop=True)
    dest = pmeta.tile([P, TC], F32, tag="dest")
    nc.vector.tensor_tensor(dest[:], cs_sb[:], ps_of[:], op=add)
    nc.vector.tensor_scalar_add(dest[:], dest[:], -1.0)
    bigt = pmeta.tile([P, TC], F32, tag="bigt")
    nc.vector.tensor_scalar(bigt[:], mask_all[:], -BIG, BIG, op0=mult, op1=add)
    nc.vector.tensor_tensor(dest[:], dest[:], bigt[:], op=add)
    dest_i = pmeta.tile([P, TC], I32, bufs=1)
    nc.vector.tensor_copy(dest_i[:], dest[:])

    # scatter metadata and token rows into compact buffers
    combc_flat = combc_d[:].rearrange("a b -> (a b)").unsqueeze(1)
    tokpos_flat = tokpos_d[:].rearrange("a b -> (a b)").unsqueeze(1)
    zf = pmeta.tile([CC, P], F32, tag="zf")
    nc.vector.memset(zf[:], 0.0)
    nc.sync.dma_start(combc_d[:], zf[:])
    zi = pmeta.tile([CC, P], I32, tag="zi")
    nc.vector.memset(zi[:], 100000)
    nc.sync.dma_start(tokpos_d[:], zi[:])
    for t in range(TC):
        nc.gpsimd.indirect_dma_start(
            out=combc_flat, out_offset=bass.IndirectOffsetOnAxis(
                ap=dest_i[:, t:t + 1], axis=0),
            in_=comb_all[:, t:t + 1], in_offset=None,
            bounds_check=C - 1, oob_is_err=False)
        nc.gpsimd.indirect_dma_start(
            out=tokpos_flat, out_offset=bass.IndirectOffsetOnAxis(
                ap=dest_i[:, t:t + 1], axis=0),
            in_=ids_sb[:, t:t + 1], in_offset=None,
            bounds_check=C - 1, oob_is_err=False)
        xsc = px.tile([P, H], BF16, tag="xsc")
        nc.sync.dma_start(xsc[:], xbf_d[t * P:(t + 1) * P, :])
        nc.gpsimd.indirect_dma_start(
            out=compact_d[:], out_offset=bass.IndirectOffsetOnAxis(
                ap=dest_i[:, t:t + 1], axis=0),
            in_=xsc[:], in_offset=None,
            bounds_check=C - 1, oob_is_err=False)

    nc.sync.dma_start(combc_sb[:], combc_d[:].rearrange("c p -> p c"))
    nc.sync.dma_start(tokpos_sb[:], tokpos_d[:].rearrange("c p -> p c"))

    ctx1.close()  # release router/compaction SBUF + PSUM

    # ---------------- expert-phase pools ----------------
    ps_g = ctx3.enter_context(tc.tile_pool(name="ps_g", bufs=2, space="PSUM"))
    ps_eo = ctx3.enter_context(tc.tile_pool(name="ps_eo", bufs=2, space="PSUM"))
    pxt = ctx3.enter_context(tc.tile_pool(name="pxt", bufs=HC))
    pwt = ctx3.enter_context(tc.tile_pool(name="pwt", bufs=2 * HC))
    pwd = ctx3.enter_context(tc.tile_pool(name="pwd", bufs=IC))
    pshw2 = ctx3.enter_context(tc.tile_pool(name="pshw2", bufs=len(msl)))

    # compact tokens transposed: xcT[h] = [128, C]
    xct = []
    for h in range(HC):
        tl = pxt.tile([P, C], BF16, tag="xct", name=f"xct{h}")
        nc.sync.dma_start(tl[:], compact_d[:, h * P:(h + 1) * P], transpose=True)
        xct.append(tl)

    # ---------------- expert GEMM1/2 on compact tokens ----------------
    def expert_gu(w_in, consumer):
        """consumer(ic, n0, nn, psum_ap) with a [128, nn] fp32 block."""
        for ic in range(IC):
            wn = pw.tile([P, H], BF16, tag="w")
            nc.gpsimd.dma_start(wn[:], w_in[ic * P:(ic + 1) * P, :])
            wts = []
            for h in range(HC):
                wt = pwt.tile([P, P], BF16, tag="wt")
                nc.sync.dma_start(wt[:], wn[:, h * P:(h + 1) * P], transpose=True)
                wts.append(wt)
            for (n0, nn) in NB_C:
                pg = ps_g.tile([P, 512], F32, space="PSUM", tag="psg")
                for h in range(HC):
                    nc.tensor.matmul(pg[:, 0:nn], wts[h][:],
                                     xct[h][:, n0:n0 + nn],
                                     start=(h == 0), stop=(h == HC - 1))
                consumer(ic, n0, nn, pg)

    sg_tiles = [psg.tile([P, C], BF16, tag="sg", name=f"sg{ic}")
                for ic in range(IC)]

    def g_consume(ic, n0, nn, pg):
        sig = pev.tile([P, 512], BF16, tag="sig")
        nc.scalar.activation(sig[:, 0:nn], pg[:, 0:nn], AF.Sigmoid)
        nc.vector.tensor_tensor(sg_tiles[ic][:, n0:n0 + nn], pg[:, 0:nn],
                                sig[:, 0:nn], op=mult)

    def u_consume(ic, n0, nn, pu):
        nc.vector.tensor_tensor(sg_tiles[ic][:, n0:n0 + nn], pu[:, 0:nn],
                                sg_tiles[ic][:, n0:n0 + nn], op=mult)

    expert_gu(wg_in, g_consume)
    expert_gu(wu_in, u_consume)

    # ---------------- shared expert GEMM3 -> out partial ----------------
    # Wsd^T tiles: [128, H] per m-chunk (rows beyond the slice are unused)
    wsdt = [pshw2.tile([P, H], BF16, tag="wsdt", name=f"wsdt{mi}")
            for mi in range(len(msl))]
    for h in range(HC):
        wn = pw.tile([P, len(msl) * P], BF16, tag="w")
        nc.vector.memset(wn[:], 0.0)
        nc.gpsimd.dma_start(wn[:, 0:ISL], wsd_in[h * P:(h + 1) * P, :])
        for mi, (m0, mm) in enumerate(msl):
            nc.sync.dma_start(wsdt[mi][:, h * P:(h + 1) * P],
                              wn[:, mi * P:(mi + 1) * P], transpose=True)

    for t in range(TC):
        ev = pev.tile([P, H], BF16, tag="shev")
        for bi, (n0, nn) in enumerate(NB_H):
            pp = ps_sh.tile([P, 512], F32, space="PSUM", tag="sh")
            for mi, (m0, mm) in enumerate(msl):
                nc.tensor.matmul(pp[:, 0:nn], smid[mi][0:mm, t * P:(t + 1) * P],
                                 wsdt[mi][0:mm, n0:n0 + nn],
                                 start=(mi == 0), stop=(mi == len(msl) - 1))
            nc.vector.tensor_copy(ev[:, n0:n0 + nn], pp[:, 0:nn])
        nc.sync.dma_start(outp_d[t * P:(t + 1) * P, :], ev[:])

    # ---------------- expert GEMM3 + combine-scale + scatter-add ----------
    wdt = [pwd.tile([P, H], BF16, tag="wdt", name=f"wdt{ic}") for ic in range(IC)]
    for h in range(HC):
        wn = pw.tile([P, II], BF16, tag="w")
        nc.gpsimd.dma_start(wn[:], wd_in[h * P:(h + 1) * P, :])
        for ic in range(IC):
            nc.sync.dma_start(wdt[ic][:, h * P:(h + 1) * P],
                              wn[:, ic * P:(ic + 1) * P], transpose=True)

    for cc in range(CC):
        ev = pev.tile([P, H], BF16, tag="eoev")
        for bi, (n0, nn) in enumerate(NB_H):
            pp = ps_eo.tile([P, 512], F32, space="PSUM", tag="pseo")
            for ic in range(IC):
                nc.tensor.matmul(pp[:, 0:nn], sg_tiles[ic][:, cc * P:(cc + 1) * P],
                                 wdt[ic][:, n0:n0 + nn],
                                 start=(ic == 0), stop=(ic == IC - 1))
            nc.vector.tensor_scalar(ev[:, n0:n0 + nn], pp[:, 0:nn],
                                    combc_sb[:, cc:cc + 1], None, op0=mult)
        nc.gpsimd.indirect_dma_start(
            out=outp_d[:], out_offset=bass.IndirectOffsetOnAxis(
                ap=tokpos_sb[:, cc:cc + 1], axis=0),
            in_=ev[:], in_offset=None,
            bounds_check=T - 1, oob_is_err=False,
            compute_op=add)

    # ---------------- reduce-scatter + output ----------------
    nc.gpsimd.collective_compute(
        "ReduceScatter", add,
        replica_groups=[list(range(cfg.NC))],
        ins=[outp_d.opt()],
        outs=[rs_d.opt()],
    )
    nc.gpsimd.dma_start(out_ext[:], rs_d[:])

    ctx3.close()
    ctx.close()


# ============================ host-side wrapper ============================

_COMPILED = {}


def _get_compiled(cfg: Cfg):
    if cfg not in _COMPILED:
        nc = bacc.Bacc("TRN2", target_bir_lowering=False, debug=False,
                       num_devices=cfg.NC)
        build_moe(nc, cfg)
        nc.compile()
        _COMPILED[cfg] = nc
    return _COMPILED[cfg]


def make_in_maps(cfg: Cfg, x, router_w, gate_w, up_w, down_w,
                 shared_gate_w, shared_up_w, shared_down_w):
    T, H, E, NC, ISL = cfg.T, cfg.H, cfg.E, cfg.NC, cfg.ISL
    xf = np.ascontiguousarray(np.asarray(x, dtype=np.float32).reshape(T, H))
    rw = np.ascontiguousarray(np.asarray(router_w, dtype=np.float32))
    in_maps = []
    for i in range(NC):
        sel = np.zeros((P, E), dtype=np.float32)
        sel[:, i] = 1.0
        in_maps.append({
            "x": xf,
            "rw": rw,
            "wg": np.ascontiguousarray(np.asarray(gate_w[i], np.float32)),
            "wu": np.ascontiguousarray(np.asarray(up_w[i], np.float32)),
            "wd": np.ascontiguousarray(np.asarray(down_w[i], np.float32)),
            "wsg": np.ascontiguousarray(
                np.asarray(shared_gate_w[i * ISL:(i + 1) * ISL], np.float32)),
            "wsu": np.ascontiguousarray(
                np.asarray(shared_up_w[i * ISL:(i + 1) * ISL], np.float32)),
            "wsd": np.ascontiguousarray(
                np.asarray(shared_down_w[:, i * ISL:(i + 1) * ISL], np.float32)),
            "sel": sel,
        })
    return in_maps


def kernel(x, router_w, gate_w, up_w, down_w,
           shared_gate_w, shared_up_w, shared_down_w, _collect=None):
    cfg = Cfg()
    B, S, H = x.shape
    assert B * S == cfg.T and H == cfg.H
    nc = _get_compiled(cfg)
    in_maps = make_in_maps(cfg, x, router_w, gate_w, up_w, down_w,
                           shared_gate_w, shared_up_w, shared_down_w)
    res = bass_utils.run_bass_kernel_spmd(nc, in_maps,
                                          core_ids=list(range(cfg.NC)))
    if _collect is not None:
        _collect.append(res)
    outs = [np.asarray(res.results[i]["out"], dtype=np.float32)
            for i in range(cfg.NC)]
    full = np.concatenate(outs, axis=0)
    return full.reshape(B, S, H)
